# revision 3
# baseline (speedup 1.0000x reference)
"""Bass/Trainium2 kernel for nn_AdvancedUpSampling2D (max-unpooling via scatter).

Full tensors in/out; internally sharded batch-parallel over 8 NeuronCores.

Key structural fact about the mask (argmax-style, include_batch_in_index=False):
  flat = (y * Wout + x) * C + c  with y = 2h + dy, x = 2w + dx, dy/dx in {0,1}
  Wout * C = 128 * 256 = 2^15, C = 2^8
  => dy = bit 15 of flat, dx = bit 8 of flat, and element (b,h,w,c) can only
     land at (b, 2h+dy, 2w+dx, c).  Windows are disjoint => no add-collisions.
So the scatter is a 4-way select + spatial interleave:
  out[b, 2h+dy', 2w+dx', c] = updates[b,h,w,c] * ((mask & 0x8100) == K(dy',dx'))

v2 design (SBUF-fabric-byte reduction):
  The 16 SBUF AXI ports (435 GB/s) carry every DMA byte on the SBUF side,
  while HBM can sustain more per-NC when reads+writes are split.  So keep
  the working set in fp16 inside SBUF and let SWDGE (gpsimd) cast on the
  DMA boundary:
    - updates load:  f32 HBM -> fp16 SBUF (8 MiB -> 4 MiB SBUF-side)
    - plane stores:  fp16 SBUF -> f32 HBM (16 MiB SBUF-side for 32 MiB HBM)
  Mask loads stay int32 on the two HWDGE rings (SP + Act), issued upfront.
  Products are written per-(dy,dx)-plane CONTIGUOUS in SBUF (keeps DVE in
  fast mode); the store DMA's DRAM-side descriptors do the 2x2 interleave
  (1 KiB runs, above the 512 B RMW threshold).
  Engines: DVE = and + 2 is_equal + 4 fp16 muls; ScalarE = 2 saturated-
  sigmoid indicators (exact 0/1) + nothing else; gpsimd = all cast DMAs.
"""

import numpy as np

# Problem config (hardcoded per contract)
B, H, W, C = 16, 64, 64, 256
SY, SX = 2, 2
N_CORES = 8
BPC = B // N_CORES          # batches per core = 2
P = 128                     # partitions = BPC * H
CW = 8                      # W-chunk per tile
NCHUNK = W // CW            # 8 chunks

_CACHE = {}


def _build_module():
    """Build the Bass module (single-core program, run SPMD on 8 cores)."""
    import concourse.bacc as bacc
    import concourse.tile as tile
    from concourse import mybir

    nc = bacc.Bacc(
        "TRN2",
        target_bir_lowering=False,
        debug=False,
        num_devices=N_CORES,
    )
    # Bias constants for the ScalarE activations (only 0.0/1.0 pre-registered).
    for v in (128.0, -32896.0):
        t = nc.alloc_sbuf_tensor(f"const-float32-{v}", [128, 1], mybir.dt.float32)
        nc.gpsimd.memset(t.ap(), v)
        nc.const_aps.aps[(mybir.dt.float32, v)] = t.ap()
    nc.all_engine_barrier()

    upd = nc.dram_tensor(
        "updates", [BPC, H, W, C], mybir.dt.float32, kind="ExternalInput"
    )
    msk = nc.dram_tensor("mask", [BPC, H, W, C], mybir.dt.int32, kind="ExternalInput")
    out = nc.dram_tensor(
        "out", [BPC, H * SY, W * SX, C], mybir.dt.float32, kind="ExternalOutput"
    )

    up_ap = upd.ap()                      # [2, 64, 64, 256]
    mk_ap = msk.ap()
    # out[b, 2h+dy, 2w+dx, c] -> view [(b h), dy, dx, w, c]
    out_v = out.ap().rearrange(
        "b (h dy) (w dx) c -> (b h) dy dx w c", dy=SY, dx=SX
    )

    # (plane key, dy, dx)
    PLANES = [
        (0x0000, 0, 0),
        (0x0100, 0, 1),
        (0x8000, 1, 0),
        (0x8100, 1, 1),
    ]

    with tile.TileContext(nc) as tc:
        with (
            tc.tile_pool(name="u", bufs=NCHUNK) as u_pool,
            tc.tile_pool(name="m", bufs=NCHUNK) as m_pool,
            tc.tile_pool(name="s", bufs=2) as s_pool,
            tc.tile_pool(name="eq", bufs=8) as eq_pool,
            tc.tile_pool(name="pl", bufs=8) as pl_pool,
        ):
            # ---- all loads upfront: no head-of-line blocking anywhere ----
            u_tiles, m_tiles = [], []
            for j in range(NCHUNK):
                w0 = j * CW
                # bufs == NCHUNK and one shared name: copy j is dedicated to
                # chunk j for the whole kernel (no WAR hazards, full lookahead)
                u_t = u_pool.tile([P, CW * C], mybir.dt.float16, name="u")
                m_t = m_pool.tile([P, CW * C], mybir.dt.int32, name="m")
                # SWDGE cast load: f32 DRAM -> fp16 SBUF
                nc.gpsimd.dma_start(
                    out=u_t[:].rearrange("p (w c) -> p w c", c=C),
                    in_=up_ap[:, :, w0 : w0 + CW, :].rearrange(
                        "b h w c -> (b h) w c"
                    ),
                )
                # mask loads on the two HWDGE rings
                dma_eng = nc.sync if j % 2 == 0 else nc.scalar
                dma_eng.dma_start(
                    out=m_t[:].rearrange("p (w c) -> p w c", c=C),
                    in_=mk_ap[:, :, w0 : w0 + CW, :].rearrange("b h w c -> (b h) w c"),
                )
                u_tiles.append(u_t)
                m_tiles.append(m_t)

            for j in range(NCHUNK):
                w0 = j * CW
                u_t, m_t = u_tiles[j], m_tiles[j]
                # s = m & 0x8100 (values in {0,256,32768,33024})
                s_t = s_pool.tile([P, CW * C], mybir.dt.int32)
                nc.vector.tensor_scalar(
                    out=s_t[:],
                    in0=m_t[:],
                    scalar1=0x8100,
                    scalar2=None,
                    op0=mybir.AluOpType.bitwise_and,
                )
                for key, dy, dx in PLANES:
                    eq = eq_pool.tile([P, CW * C], mybir.dt.float16, name="eq")
                    if key == 0x0000:
                        # s==0 <=> s<128: saturated step, one ScalarE op.
                        nc.scalar.activation(
                            eq[:],
                            s_t[:],
                            mybir.ActivationFunctionType.Sigmoid,
                            bias=128.0,
                            scale=-1.0,
                        )
                    elif key == 0x8100:
                        # s==33024 <=> s>32896: saturated step, one ScalarE op
                        nc.scalar.activation(
                            eq[:],
                            s_t[:],
                            mybir.ActivationFunctionType.Sigmoid,
                            bias=-32896.0,
                            scale=1.0,
                        )
                    else:
                        # middle keys: exact is_equal on DVE
                        nc.vector.tensor_scalar(
                            out=eq[:],
                            in0=s_t[:],
                            scalar1=key,
                            scalar2=None,
                            op0=mybir.AluOpType.is_equal,
                        )
                    # contiguous fp16 plane product (dense -> DVE 2x mode)
                    pl = pl_pool.tile([P, CW * C], mybir.dt.float16, name="pl")
                    nc.vector.tensor_mul(out=pl[:], in0=u_t[:], in1=eq[:])
                    # SWDGE cast store: fp16 SBUF -> f32 DRAM quadrant
                    nc.gpsimd.dma_start(
                        out=out_v[:, dy, dx, w0 : w0 + CW, :],
                        in_=pl[:].rearrange("p (w c) -> p w c", c=C),
                    )
    nc.finalize()
    return nc


def _get_nc():
    if "nc" not in _CACHE:
        _CACHE["nc"] = _build_module()
    return _CACHE["nc"]


def _get_runner():
    """Cached jitted shard_map executable (run_bass_via_pjrt rebuilds its jit
    closure per call, reloading the executable each time; this caches it)."""
    if "runner" in _CACHE:
        return _CACHE["runner"]
    import jax
    import jax.numpy as jnp
    from jax.experimental.shard_map import shard_map
    from jax.sharding import Mesh, PartitionSpec

    import concourse.mybir as mybir
    from concourse import bass2jax

    nc = _get_nc()
    bass2jax.install_neuronx_cc_hook()

    partition_name = nc.partition_id_tensor.name if nc.partition_id_tensor else None
    in_names, out_names, out_avals = [], [], []
    for alloc in nc.m.functions[0].allocations:
        if not isinstance(alloc, mybir.MemoryLocationSet):
            continue
        name = alloc.memorylocations[0].name
        if alloc.kind == "ExternalInput":
            if name != partition_name:
                in_names.append(name)
        elif alloc.kind == "ExternalOutput":
            out_names.append(name)
            out_avals.append(
                jax.core.ShapedArray(
                    tuple(alloc.tensor_shape), mybir.dt.np(alloc.dtype)
                )
            )
    n_params = len(in_names)
    n_outs = len(out_names)
    all_names = [*in_names, *out_names]
    if partition_name is not None:
        all_names.append(partition_name)

    def _body(*args):
        operands = list(args)
        if partition_name is not None:
            operands.append(bass2jax.partition_id_tensor())
        outs = bass2jax._bass_exec_p.bind(
            *operands,
            out_avals=tuple(out_avals),
            in_names=tuple(all_names),
            out_names=tuple(out_names),
            lowering_input_output_aliases=(),
            sim_require_finite=True,
            sim_require_nnan=True,
            nc=nc,
        )
        return tuple(outs)

    devices = jax.devices()[:N_CORES]
    mesh = Mesh(np.asarray(devices), ("core",))
    sharded = jax.jit(
        shard_map(
            _body,
            mesh=mesh,
            in_specs=(PartitionSpec("core"),) * (n_params + n_outs),
            out_specs=(PartitionSpec("core"),) * n_outs,
            check_rep=False,
        ),
        donate_argnums=tuple(range(n_params, n_params + n_outs)),
        keep_unused=True,
    )
    # Donated output buffers made on-device (no host->device zero transfer).
    zero_makers = [
        jax.jit(
            lambda shape=tuple(a.shape), dtype=a.dtype: jnp.zeros(
                (N_CORES * shape[0], *shape[1:]), dtype
            )
        )
        for a in out_avals
    ]

    def run(updates, mask):
        ins = {"updates": updates, "mask": mask}
        out_arrs = sharded(
            *[ins[name] for name in in_names], *[mk() for mk in zero_makers]
        )
        return np.asarray(out_arrs[out_names.index("out")])

    _CACHE["runner"] = run
    return run


def _run(updates, mask, trace=False):
    updates = np.ascontiguousarray(updates, dtype=np.float32)
    mask = np.ascontiguousarray(mask, dtype=np.int32)

    if not trace:
        return _get_runner()(updates, mask), None

    # Profiling path (test.py): go through the library so NTFF capture works.
    from concourse.bass_utils import run_bass_kernel_spmd

    nc = _get_nc()
    in_maps = [
        {
            "updates": updates[i * BPC : (i + 1) * BPC],
            "mask": mask[i * BPC : (i + 1) * BPC],
        }
        for i in range(N_CORES)
    ]
    res = run_bass_kernel_spmd(
        nc,
        in_maps,
        core_ids=list(range(N_CORES)),
        trace=trace,
    )
    out = np.concatenate([r["out"] for r in res.results], axis=0)
    return out, res


def kernel(**inputs):
    out, _ = _run(inputs["updates"], inputs["mask"])
    return out
